# revision 20
# baseline (speedup 1.0000x reference)
"""Trainium2 Bass kernel for the moe_routing Adapter module.

Computes, SPMD over 8 NeuronCores (batch-sharded):
  x_mean  = mean_L(x); sim = cos(x_mean, adapter_key)  -> per-sample argmax
  majority vote over the 10-bin count vector (one small AllReduce)
  bias = x + relu(relu(x @ W1[major].T) @ W2[major].T);  reduce_sim; idx

kernel(**inputs) takes FULL inputs, returns (bias, reduce_sim, idx) like the
reference. Self-contained: only imports concourse (on PYTHONPATH) + numpy.

Implementation notes:
- x is cast-DMA'd f32->bf16 into SBUF once (natural layout) and kept resident;
  output writes stream back.  Total HBM traffic/core ~42MB (memory-bound).
- Per-sample means contract over rows, done as PE matmuls against per-tile
  0/1 indicator matrices (no transpose needed before the vote/AllReduce).
- x.T tiles (needed because mm1 contracts over the feature dim) are produced
  by PE transpose-mode chunk-by-chunk, overlapped with the AllReduce + mm1.
- No registers/dynamic DMA (value_load crashes this runtime): all 10 adapters
  are preloaded + pre-transposed; W[major] is selected arithmetically with a
  one-hot vector broadcast across partitions via a K=1 outer-product matmul.
"""

from contextlib import ExitStack

import numpy as np

import concourse.bass as bass
import concourse.bacc as bacc
import concourse.tile as tile
import concourse.mybir as mybir
from concourse.bass import ds, ts
from concourse.bass_utils import run_bass_kernel_spmd
from concourse.masks import make_identity

N_CORES = 8
B, L, C = 256, 197, 768
H = 48
NA = 10
NCH = C // 128  # 6 column chunks

F32 = mybir.dt.float32
BF16 = mybir.dt.bfloat16
I32 = mybir.dt.int32
AX = mybir.AxisListType
ALU = mybir.AluOpType
ACTF = mybir.ActivationFunctionType


def build(spc=B // N_CORES):
    """Build the SPMD Bass program for `spc` samples per core."""
    rows = spc * L
    nt = (rows + 127) // 128          # row tiles
    rpad = nt * 128
    tail = rows - (nt - 1) * 128      # valid rows in last tile
    chunks = [(o, min(512, rpad - o)) for o in range(0, rpad, 512)]

    nc = bacc.Bacc("TRN2", target_bir_lowering=False, debug=False,
                   num_devices=N_CORES)

    x_ext = nc.dram_tensor("x_embed", [rows, C], F32, kind="ExternalInput").ap()
    w1_ext = nc.dram_tensor("W1", [NA, H, C], F32, kind="ExternalInput").ap()
    w2_ext = nc.dram_tensor("W2", [NA, C, H], F32, kind="ExternalInput").ap()
    key_ext = nc.dram_tensor("adapter_key", [NA, C], F32, kind="ExternalInput").ap()
    out_ext = nc.dram_tensor("out", [rows, C], F32, kind="ExternalOutput").ap()
    sc_ext = nc.dram_tensor("scalars", [1, 2], F32, kind="ExternalOutput").ap()

    with tile.TileContext(nc) as tc:
        with (
            tc.tile_pool(name="big", bufs=1) as big,
            tc.tile_pool(name="small", bufs=1) as small,
            tc.tile_pool(name="ps_t", bufs=2, space="PSUM") as ps_t,
            tc.tile_pool(name="dram", bufs=1, space="DRAM") as dram,
        ):
            # ---- persistent SBUF tensors ----
            x_nat = big.tile([128, nt, C], BF16)        # natural layout, bf16
            w1Ta = big.tile([128, NA, NCH, H], BF16)    # W1[a].T chunks
            v2a = big.tile([H, NA, C], BF16)            # W2[a].T
            ident = small.tile([128, 128], BF16)
            ident10 = small.tile([NA, NA], F32)
            ident_s = small.tile([spc, spc], F32)
            make_identity(nc, ident)
            make_identity(nc, ident10)
            make_identity(nc, ident_s)

            # ---- load x (cast f32->bf16 via SWDGE), ~10 DMAs ----
            ntf = rows // 128  # full tiles
            group = max(1, (ntf + 9) // 10)
            for g0 in range(0, ntf, group):
                g1 = min(g0 + group, ntf)
                nc.gpsimd.dma_start(
                    out=x_nat[:, g0:g1, :],
                    in_=x_ext[g0 * 128:g1 * 128, :].rearrange(
                        "(t p) c -> p t c", p=128),
                )
            if tail < 128:
                nc.gpsimd.memset(x_nat[:, nt - 1, :], 0.0)
                nc.gpsimd.dma_start(
                    out=x_nat[0:tail, nt - 1, :],
                    in_=x_ext[(nt - 1) * 128:rows, :],
                )

            # ---- per-tile sample indicator matrices E[r, s] ----
            E = small.tile([128, nt, spc], BF16)
            nc.gpsimd.memset(E[:], 1.0)
            for t in range(nt):
                nc.gpsimd.affine_select(
                    out=E[:, t, :], in_=E[:, t, :],
                    pattern=[[-L, spc]], base=128 * t, channel_multiplier=1,
                    compare_op=ALU.is_ge, fill=0.0)
                nc.gpsimd.affine_select(
                    out=E[:, t, :], in_=E[:, t, :],
                    pattern=[[L, spc]], base=(L - 1) - 128 * t,
                    channel_multiplier=-1,
                    compare_op=ALU.is_ge, fill=0.0)

            # ---- keys: load + l2-normalize + transpose ----
            keys = small.tile([NA, C], F32)
            nc.sync.dma_start(out=keys[:], in_=key_ext[:])
            ksq = small.tile([NA, C], F32)
            nc.vector.tensor_tensor(out=ksq[:], in0=keys[:], in1=keys[:],
                                    op=ALU.mult)
            kss = small.tile([NA, 1], F32)
            nc.vector.tensor_reduce(out=kss[:], in_=ksq[:], axis=AX.X, op=ALU.add)
            nc.vector.tensor_scalar_max(kss[:], kss[:], 1e-12)
            krt = small.tile([NA, 1], F32)
            nc.scalar.sqrt(krt[:], kss[:])
            kri = small.tile([NA, 1], F32)
            nc.vector.reciprocal(kri[:], krt[:])
            keyn = small.tile([NA, C], F32)
            nc.vector.tensor_scalar_mul(keyn[:], keys[:], kri[:])
            keynT = small.tile([128, NCH, NA], F32)
            for ci in range(NCH):
                ktp = ps_t.tile([128, NA], F32, tag="tp", bufs=2)
                nc.tensor.transpose(ktp[:], keyn[:, ts(ci, 128)], ident10[:])
                nc.vector.tensor_copy(keynT[:, ci, :], ktp[:])

            # ---- all-10 adapter weights: load (cast bf16) + transpose ----
            with tc.tile_pool(name="w1p", bufs=1) as w1pool:
                w1a = w1pool.tile([H, NA, C], BF16)
                nc.gpsimd.dma_start(out=w1a[:],
                                    in_=w1_ext.rearrange("a h c -> h a c"))
                for a in range(NA):
                    for ci in range(NCH):
                        wtp = ps_t.tile([128, 128], BF16, tag="tp", bufs=2)
                        nc.tensor.transpose(wtp[:, 0:H], w1a[:, a, ts(ci, 128)],
                                            ident[0:H, 0:H])
                        nc.vector.tensor_copy(w1Ta[:, a, ci, :], wtp[:, 0:H])
            with tc.tile_pool(name="w2p", bufs=1) as w2pool:
                w2a = w2pool.tile([128, NA, NCH, H], BF16)
                for a in range(NA):
                    nc.gpsimd.dma_start(
                        out=w2a[:, a, :, :],
                        in_=w2_ext[a, :, :].rearrange("(q p) h -> p q h", p=128))
                for a in range(NA):
                    for ci in range(NCH):
                        vtp = ps_t.tile([128, 128], BF16, tag="tp", bufs=2)
                        nc.tensor.transpose(vtp[0:H, :], w2a[:, a, ci, :],
                                            ident[:])
                        nc.vector.tensor_copy(v2a[:, a, ts(ci, 128)],
                                              vtp[0:H, :])

            with (
                tc.tile_pool(name="ps_m", bufs=1, space="PSUM") as ps_m_pool,
                tc.tile_pool(name="ps_s", bufs=1, space="PSUM") as ps_s,
            ):
                # ---- per-sample sums: xsum[s, c] = sum_r E[r,s] x[r,c] ----
                ps_m = ps_m_pool.tile([spc, C], F32)
                for t in range(nt):
                    nc.tensor.matmul(ps_m[:, 0:512], lhsT=E[:, t, :],
                                     rhs=x_nat[:, t, 0:512],
                                     start=(t == 0), stop=(t == nt - 1))
                for t in range(nt):
                    nc.tensor.matmul(ps_m[:, 512:C], lhsT=E[:, t, :],
                                     rhs=x_nat[:, t, 512:C],
                                     start=(t == 0), stop=(t == nt - 1))
                xsum = small.tile([spc, C], F32)
                nc.vector.tensor_copy(xsum[:], ps_m[:])

                # ---- ||xsum||^2, 1/||xsum|| ----
                xsq = small.tile([spc, C], F32)
                nc.vector.tensor_tensor(out=xsq[:], in0=xsum[:], in1=xsum[:],
                                        op=ALU.mult)
                ssq = small.tile([spc, 1], F32)
                nc.vector.tensor_reduce(out=ssq[:], in_=xsq[:], axis=AX.X,
                                        op=ALU.add)
                nc.vector.tensor_scalar_max(ssq[:], ssq[:], 1e-9)
                srt = small.tile([spc, 1], F32)
                nc.scalar.sqrt(srt[:], ssq[:])
                sri = small.tile([spc, 1], F32)
                nc.vector.reciprocal(sri[:], srt[:])

                # ---- xsum.T chunks, then sim = (xsum @ keyn.T) / ||xsum|| ----
                xsumT = small.tile([128, NCH, spc], F32)
                for ci in range(NCH):
                    stp = ps_t.tile([128, spc], F32, tag="tp", bufs=2)
                    nc.tensor.transpose(stp[:], xsum[:, ts(ci, 128)], ident_s[:])
                    nc.vector.tensor_copy(xsumT[:, ci, :], stp[:])
                ps_sim = ps_s.tile([spc, NA], F32, tag="sim")
                for ci in range(NCH):
                    nc.tensor.matmul(ps_sim[:], lhsT=xsumT[:, ci, :],
                                     rhs=keynT[:, ci, :],
                                     start=(ci == 0), stop=(ci == NCH - 1))
                sim = small.tile([spc, NA], F32)
                nc.vector.tensor_scalar_mul(sim[:], ps_sim[:], sri[:])

                # ---- per-sample argmax (first max) -> one-hot ----
                smx = small.tile([spc, 1], F32)
                nc.vector.tensor_reduce(out=smx[:], in_=sim[:], axis=AX.X,
                                        op=ALU.max)
                mask = small.tile([spc, NA], I32)
                nc.vector.tensor_scalar(out=mask[:], in0=sim[:], scalar1=smx[:],
                                        scalar2=None, op0=ALU.is_ge)
                iota_s = small.tile([spc, NA], I32)
                nc.gpsimd.iota(iota_s[:], pattern=[[1, NA]], base=0,
                               channel_multiplier=0)
                iota_f = small.tile([spc, NA], F32)
                nc.vector.tensor_copy(iota_f[:], iota_s[:])
                bigc = small.tile([spc, NA], I32)
                nc.vector.memset(bigc[:], 127)
                msel = small.tile([spc, NA], I32)
                nc.vector.select(msel[:], mask[:], iota_s[:], bigc[:])
                psidx = small.tile([spc, 1], I32)
                nc.vector.tensor_reduce(out=psidx[:], in_=msel[:], axis=AX.X,
                                        op=ALU.min)
                psidx_f = small.tile([spc, 1], F32)
                nc.vector.tensor_copy(psidx_f[:], psidx[:])
                oh = small.tile([spc, NA], F32)
                nc.vector.tensor_scalar(out=oh[:], in0=iota_f[:],
                                        scalar1=psidx_f[:],
                                        scalar2=None, op0=ALU.is_equal)

                # ---- counts / colsum -> [1, 20] ----
                ones = small.tile([128, 1], F32)
                nc.vector.memset(ones[:], 1.0)
                ps_cnt = ps_s.tile([1, 2 * NA], F32, tag="cnt")
                nc.tensor.matmul(ps_cnt[:, 0:NA], lhsT=ones[0:spc, :], rhs=oh[:],
                                 start=True, stop=True)
                nc.tensor.matmul(ps_cnt[:, NA:2 * NA], lhsT=ones[0:spc, :],
                                 rhs=sim[:], start=True, stop=True)
                ar_in = small.tile([1, 2 * NA], F32)
                nc.vector.tensor_copy(ar_in[:], ps_cnt[:])

            # ---- AllReduce of [1, 20] ----
            cc_in = dram.tile([1, 2 * NA], F32)
            cc_out = dram.tile([1, 2 * NA], F32, addr_space="Shared")
            nc.gpsimd.dma_start(cc_in[:], ar_in[:])
            nc.gpsimd.collective_compute(
                "AllReduce", ALU.add,
                replica_groups=[list(range(N_CORES))],
                ins=[cc_in.opt()], outs=[cc_out.opt()],
            )
            ar = small.tile([1, 2 * NA], F32)
            nc.sync.dma_start(out=ar[:], in_=cc_out[:])

            # ---- majority vote (first max) + reduce_sim ----
            cmx = small.tile([1, 1], F32)
            nc.vector.tensor_reduce(out=cmx[:], in_=ar[:, 0:NA], axis=AX.X,
                                    op=ALU.max)
            cmask = small.tile([1, NA], I32)
            nc.vector.tensor_scalar(out=cmask[:], in0=ar[:, 0:NA], scalar1=cmx[:],
                                    scalar2=None, op0=ALU.is_ge)
            iota1 = small.tile([1, NA], I32)
            nc.gpsimd.iota(iota1[:], pattern=[[1, NA]], base=0,
                           channel_multiplier=0)
            iota1f = small.tile([1, NA], F32)
            nc.vector.tensor_copy(iota1f[:], iota1[:])
            big1 = small.tile([1, NA], I32)
            nc.vector.memset(big1[:], 127)
            csel = small.tile([1, NA], I32)
            nc.vector.select(csel[:], cmask[:], iota1[:], big1[:])
            major = small.tile([1, 1], I32)
            nc.vector.tensor_reduce(out=major[:], in_=csel[:], axis=AX.X,
                                    op=ALU.min)
            major_f = small.tile([1, 1], F32)
            nc.vector.tensor_copy(major_f[:], major[:])
            ohm = small.tile([1, NA], F32)
            nc.vector.tensor_scalar(out=ohm[:], in0=iota1f[:], scalar1=major_f[:],
                                    scalar2=None, op0=ALU.is_equal)
            rsp = small.tile([1, NA], F32)
            nc.vector.tensor_tensor(out=rsp[:], in0=ohm[:], in1=ar[:, NA:2 * NA],
                                    op=ALU.mult)
            rss = small.tile([1, 1], F32)
            nc.vector.tensor_reduce(out=rss[:], in_=rsp[:], axis=AX.X, op=ALU.add)
            scal = small.tile([1, 2], F32)
            nc.vector.tensor_copy(scal[:, 0:1], major_f[:])
            nc.vector.tensor_scalar_mul(scal[:, 1:2], rss[:],
                                        1.0 / (spc * N_CORES))
            nc.sync.dma_start(out=sc_ext[:], in_=scal[:])

            # ---- select W[major] arithmetically via one-hot ----
            with tc.tile_pool(name="ps_ob", bufs=1, space="PSUM") as ps_ob:
                ones_r = small.tile([1, 128], F32)
                nc.vector.memset(ones_r[:], 1.0)
                ohm_ps = ps_ob.tile([128, NA], F32)
                nc.tensor.matmul(ohm_ps[:], lhsT=ones_r[:], rhs=ohm[:],
                                 start=True, stop=True)
                ohm_b = small.tile([128, NA], F32)
                nc.vector.tensor_copy(ohm_b[:], ohm_ps[:])

                w1T = small.tile([128, NCH, H], BF16)
                wt1 = small.tile([128, NCH, H], BF16)
                nc.vector.tensor_scalar_mul(w1T[:], w1Ta[:, 0, :, :],
                                            ohm_b[:, 0:1])
                for a in range(1, NA):
                    nc.vector.tensor_scalar_mul(wt1[:], w1Ta[:, a, :, :],
                                                ohm_b[:, a:a + 1])
                    nc.vector.tensor_tensor(out=w1T[:], in0=w1T[:], in1=wt1[:],
                                            op=ALU.add)
                v2 = small.tile([H, C], BF16)
                vt1 = small.tile([H, C], BF16)
                nc.vector.tensor_scalar_mul(v2[:], v2a[:, 0, :], ohm_b[0:H, 0:1])
                for a in range(1, NA):
                    nc.vector.tensor_scalar_mul(vt1[:], v2a[:, a, :],
                                                ohm_b[0:H, a:a + 1])
                    nc.vector.tensor_tensor(out=v2[:], in0=v2[:], in1=vt1[:],
                                            op=ALU.add)

                # ---- main pass, chunk-streamed:
                #      transpose x -> mm1 -> relu -> mm2 -> relu+residual ----
                with (
                    tc.tile_pool(name="xtc", bufs=4) as xtc,
                    tc.tile_pool(name="htc", bufs=2) as htc,
                    tc.tile_pool(name="ps_h", bufs=1, space="PSUM") as ps_h,
                    tc.tile_pool(name="ps_a", bufs=2, space="PSUM") as ps_a,
                    tc.tile_pool(name="ost", bufs=1) as ost,
                ):
                    for k, (off, n) in enumerate(chunks):
                        nsub = n // 128
                        xTc = xtc.tile([128, NCH, 512], BF16, tag="xtc")
                        for j in range(nsub):
                            t = off // 128 + j
                            for ci in range(NCH):
                                xtp = ps_t.tile([128, 128], BF16, tag="tp",
                                                bufs=2)
                                nc.tensor.transpose(
                                    xtp[:], x_nat[:, t, ts(ci, 128)], ident[:])
                                nc.vector.tensor_copy(
                                    xTc[:, ci, ts(j, 128)], xtp[:])
                        ph = ps_h.tile([H, 512], F32, tag="ph")
                        for ci in range(NCH):
                            nc.tensor.matmul(ph[:, 0:n], lhsT=w1T[:, ci, :],
                                             rhs=xTc[:, ci, 0:n],
                                             start=(ci == 0),
                                             stop=(ci == NCH - 1))
                        ht = htc.tile([H, 512], BF16, tag="ht")
                        nc.scalar.activation(out=ht[:, 0:n], in_=ph[:, 0:n],
                                             func=ACTF.Relu)
                        for j in range(nsub):
                            t = off // 128 + j
                            pa = ps_a.tile([128, C], F32, tag="pa")
                            nc.tensor.matmul(pa[:, 0:512],
                                             lhsT=ht[:, ts(j, 128)],
                                             rhs=v2[:, 0:512],
                                             start=True, stop=True)
                            nc.tensor.matmul(pa[:, 512:C],
                                             lhsT=ht[:, ts(j, 128)],
                                             rhs=v2[:, 512:C],
                                             start=True, stop=True)
                            rel = ost.tile([128, C], F32, tag="rel", bufs=3)
                            nc.scalar.activation(out=rel[:], in_=pa[:],
                                                 func=ACTF.Relu)
                            ob = ost.tile([128, C], F32, tag="ob", bufs=3)
                            nc.vector.tensor_tensor(out=ob[:], in0=rel[:],
                                                    in1=x_nat[:, t, :],
                                                    op=ALU.add)
                            nr = 128 if t < nt - 1 else tail
                            nc.sync.dma_start(
                                out=out_ext[t * 128:t * 128 + nr, :],
                                in_=ob[0:nr, :])

    nc.compile()
    return nc


_built = {}


def _get_nc(spc):
    if spc not in _built:
        _built[spc] = build(spc)
    return _built[spc]


def kernel(x_embed, W1, W2, adapter_key):
    x_embed = np.ascontiguousarray(x_embed, dtype=np.float32)
    W1 = np.ascontiguousarray(W1, dtype=np.float32)
    W2 = np.ascontiguousarray(W2, dtype=np.float32)
    adapter_key = np.ascontiguousarray(adapter_key, dtype=np.float32)

    spc = B // N_CORES
    rows = spc * L
    nc = _get_nc(spc)
    shards = x_embed.reshape(N_CORES, rows, C)
    in_maps = [
        {"x_embed": shards[i], "W1": W1, "W2": W2, "adapter_key": adapter_key}
        for i in range(N_CORES)
    ]
    res = run_bass_kernel_spmd(nc, in_maps, list(range(N_CORES)))
    bias = np.concatenate(
        [res.results[i]["out"].reshape(1, spc, L, C) for i in range(N_CORES)],
        axis=0).reshape(B, L, C)
    scal = res.results[0]["scalars"]
    major = int(round(float(scal[0, 0])))
    reduce_sim = np.float32(scal[0, 1])
    idx = np.full((B, 1), major, dtype=np.int32)
    return bias, np.asarray(reduce_sim), idx


# revision 32
# speedup vs baseline: 2.2119x; 2.2119x over previous
"""Trainium2 Bass kernel for the moe_routing Adapter module.

Computes, SPMD over 8 NeuronCores (batch-sharded):
  x_mean  = mean_L(x); sim = cos(x_mean, adapter_key)  -> per-sample argmax
  majority vote over the 10-bin count vector (one small AllReduce)
  bias = x + relu(relu(x @ W1[major].T) @ W2[major].T);  reduce_sim; idx

kernel(**inputs) takes FULL inputs, returns (bias, reduce_sim, idx) like the
reference. Self-contained: only imports concourse (on PYTHONPATH) + numpy.

Implementation notes:
- x is cast-DMA'd f32->bf16 into SBUF once (natural layout) and kept resident;
  output writes stream back.  Total HBM traffic/core ~42MB (memory-bound).
- Per-sample means contract over rows, done as PE matmuls against per-tile
  0/1 indicator matrices (no transpose needed before the vote/AllReduce).
- x.T tiles (needed because mm1 contracts over the feature dim) are produced
  by PE transpose-mode chunk-by-chunk, overlapped with the AllReduce + mm1.
- No registers/dynamic DMA (value_load crashes this runtime): all 10 adapters
  are preloaded + pre-transposed; W[major] is selected arithmetically with a
  one-hot vector broadcast across partitions via a K=1 outer-product matmul.
"""

from contextlib import ExitStack

import numpy as np

import concourse.bass as bass
import concourse.bacc as bacc
import concourse.tile as tile
import concourse.mybir as mybir
from concourse.bass import ds, ts
from concourse.bass_utils import run_bass_kernel_spmd
from concourse.masks import make_identity
from concourse.tile_rust import add_dep_helper

N_CORES = 8
B, L, C = 256, 197, 768
H = 48
NA = 10
NCH = C // 128  # 6 column chunks

F32 = mybir.dt.float32
BF16 = mybir.dt.bfloat16
I32 = mybir.dt.int32
AX = mybir.AxisListType
ALU = mybir.AluOpType
ACTF = mybir.ActivationFunctionType


def _emit_body(nc, tc, spc, no_ar, x_ext, w1_ext, w2_ext, key_ext,
               out_ext, sc_ext):
    rows = spc * L
    nt = (rows + 127) // 128          # row tiles
    rpad = nt * 128
    tail = rows - (nt - 1) * 128      # valid rows in last tile
    chunks = [(o, min(512, rpad - o)) for o in range(0, rpad, 512)]

    if True:
        with (
            tc.tile_pool(name="big", bufs=1) as big,
            tc.tile_pool(name="small", bufs=1) as small,
            tc.tile_pool(name="ps_t", bufs=2, space="PSUM") as ps_t,
            tc.tile_pool(name="dram", bufs=1, space="DRAM") as dram,
        ):
            # ---- persistent SBUF tensors ----
            x_nat = big.tile([128, nt, C], BF16)        # natural layout, bf16
            w1Ta = big.tile([128, NA, NCH, H], BF16)    # W1[a].T chunks
            ident = small.tile([128, 128], BF16)
            ident10 = small.tile([NA, NA], F32)
            ident_s = small.tile([spc, spc], F32)
            make_identity(nc, ident)
            make_identity(nc, ident10)
            make_identity(nc, ident_s)

            # ---- all-10 adapter weights (issued before x: small, feeds
            #      PE transposes early; single contiguous cast DMAs) ----
            # W2[a] flattened: partition p holds rows 6p..6p+5 (c = 6p+q);
            # selected post-AR, then transposed (6 PE transposes).
            w2n = big.tile([128, NA, 6 * H], BF16)
            nc.gpsimd.dma_start(
                out=w2n[:],
                in_=w2_ext.rearrange("a (p q) h -> p a (q h)", p=128))
            with tc.tile_pool(name="wsta", bufs=1) as wsta:
                w1a = wsta.tile([H, NA, C], BF16)
                nc.gpsimd.dma_start(out=w1a[:],
                                    in_=w1_ext.rearrange("a h c -> h a c"))
                for a in range(NA):
                    wtp = ps_t.tile([128, NCH * H], BF16, tag="tp", bufs=2)
                    for ci in range(NCH):
                        nc.tensor.transpose(wtp[:, ts(ci, H)],
                                            w1a[:, a, ts(ci, 128)],
                                            ident[0:H, 0:H])
                    nc.vector.tensor_copy(
                        w1Ta[:, a, :, :].rearrange("p x h -> p (x h)"), wtp[:])

            # ---- load x (cast f32->bf16 via SWDGE), ~10 DMAs ----
            ntf = rows // 128  # full tiles
            group = max(1, (ntf + 9) // 10)
            for g0 in range(0, ntf, group):
                g1 = min(g0 + group, ntf)
                nc.gpsimd.dma_start(
                    out=x_nat[:, g0:g1, :],
                    in_=x_ext[g0 * 128:g1 * 128, :].rearrange(
                        "(t p) c -> p t c", p=128),
                )
            if tail < 128:
                nc.gpsimd.memset(x_nat[:, nt - 1, :], 0.0)
                nc.gpsimd.dma_start(
                    out=x_nat[0:tail, nt - 1, :],
                    in_=x_ext[(nt - 1) * 128:rows, :],
                )

            # ---- per-tile sample indicator matrices E[r, s] ----
            E = small.tile([128, nt, spc], BF16)
            nc.gpsimd.memset(E[:], 1.0)
            nc.gpsimd.affine_select(
                out=E[:], in_=E[:],
                pattern=[[128, nt], [-L, spc]], base=0, channel_multiplier=1,
                compare_op=ALU.is_ge, fill=0.0)
            nc.gpsimd.affine_select(
                out=E[:], in_=E[:],
                pattern=[[-128, nt], [L, spc]], base=L - 1,
                channel_multiplier=-1,
                compare_op=ALU.is_ge, fill=0.0)

            # ---- keys: load + l2-normalize + transpose ----
            keys = small.tile([NA, C], F32)
            nc.sync.dma_start(out=keys[:], in_=key_ext[:])
            ksq = small.tile([NA, C], F32)
            nc.vector.tensor_tensor(out=ksq[:], in0=keys[:], in1=keys[:],
                                    op=ALU.mult)
            kss = small.tile([NA, 1], F32)
            nc.vector.tensor_reduce(out=kss[:], in_=ksq[:], axis=AX.X, op=ALU.add)
            nc.vector.tensor_scalar_max(kss[:], kss[:], 1e-12)
            krt = small.tile([NA, 1], F32)
            nc.scalar.sqrt(krt[:], kss[:])
            kri = small.tile([NA, 1], F32)
            nc.vector.reciprocal(kri[:], krt[:])
            keyn = small.tile([NA, C], F32)
            nc.vector.tensor_scalar_mul(keyn[:], keys[:], kri[:])
            keynT = small.tile([128, NCH, NA], F32)
            for ci in range(NCH):
                ktp = ps_t.tile([128, NA], F32, tag="tp", bufs=2)
                nc.tensor.transpose(ktp[:], keyn[:, ts(ci, 128)], ident10[:])
                nc.vector.tensor_copy(keynT[:, ci, :], ktp[:])

            # ---- all-10 adapter weights: load (cast bf16) + transpose ----

            with (
                tc.tile_pool(name="ps_m", bufs=1, space="PSUM") as ps_m_pool,
                tc.tile_pool(name="ps_s", bufs=1, space="PSUM") as ps_s,
            ):
                # ---- per-sample sums: xsum[s, c] = sum_r E[r,s] x[r,c] ----
                ps_m = ps_m_pool.tile([spc, C], F32)
                for t in range(nt):
                    nc.tensor.matmul(ps_m[:, 0:512], lhsT=E[:, t, :],
                                     rhs=x_nat[:, t, 0:512],
                                     start=(t == 0), stop=(t == nt - 1))
                for t in range(nt):
                    nc.tensor.matmul(ps_m[:, 512:C], lhsT=E[:, t, :],
                                     rhs=x_nat[:, t, 512:C],
                                     start=(t == 0), stop=(t == nt - 1))
                xsum = small.tile([spc, C], F32)
                nc.vector.tensor_copy(xsum[:], ps_m[:])

                # ---- ||xsum||^2, 1/||xsum|| ----
                xsq = small.tile([spc, C], F32)
                nc.vector.tensor_tensor(out=xsq[:], in0=xsum[:], in1=xsum[:],
                                        op=ALU.mult)
                ssq = small.tile([spc, 1], F32)
                nc.vector.tensor_reduce(out=ssq[:], in_=xsq[:], axis=AX.X,
                                        op=ALU.add)
                nc.vector.tensor_scalar_max(ssq[:], ssq[:], 1e-9)
                srt = small.tile([spc, 1], F32)
                nc.scalar.sqrt(srt[:], ssq[:])
                sri = small.tile([spc, 1], F32)
                nc.vector.reciprocal(sri[:], srt[:])

                # ---- xsum.T chunks, then sim = (xsum @ keyn.T) / ||xsum|| ----
                xsumT = small.tile([128, NCH, spc], F32)
                for ci in range(NCH):
                    stp = ps_t.tile([128, spc], F32, tag="tp", bufs=2)
                    nc.tensor.transpose(stp[:], xsum[:, ts(ci, 128)], ident_s[:])
                    nc.vector.tensor_copy(xsumT[:, ci, :], stp[:])
                ps_sim = ps_s.tile([spc, NA], F32, tag="sim")
                for ci in range(NCH):
                    nc.tensor.matmul(ps_sim[:], lhsT=xsumT[:, ci, :],
                                     rhs=keynT[:, ci, :],
                                     start=(ci == 0), stop=(ci == NCH - 1))
                sim = small.tile([spc, NA], F32)
                nc.vector.tensor_scalar_mul(sim[:], ps_sim[:], sri[:])

                # ---- per-sample argmax (first max) -> one-hot ----
                smx = small.tile([spc, 1], F32)
                nc.vector.tensor_reduce(out=smx[:], in_=sim[:], axis=AX.X,
                                        op=ALU.max)
                mask = small.tile([spc, NA], I32)
                nc.vector.tensor_scalar(out=mask[:], in0=sim[:], scalar1=smx[:],
                                        scalar2=None, op0=ALU.is_ge)
                iota_s = small.tile([spc, NA], I32)
                nc.gpsimd.iota(iota_s[:], pattern=[[1, NA]], base=0,
                               channel_multiplier=0)
                iota_f = small.tile([spc, NA], F32)
                nc.vector.tensor_copy(iota_f[:], iota_s[:])
                bigc = small.tile([spc, NA], I32)
                nc.vector.memset(bigc[:], 127)
                msel = small.tile([spc, NA], I32)
                nc.vector.select(msel[:], mask[:], iota_s[:], bigc[:])
                psidx = small.tile([spc, 1], I32)
                nc.vector.tensor_reduce(out=psidx[:], in_=msel[:], axis=AX.X,
                                        op=ALU.min)
                psidx_f = small.tile([spc, 1], F32)
                nc.vector.tensor_copy(psidx_f[:], psidx[:])
                oh = small.tile([spc, NA], F32)
                nc.vector.tensor_scalar(out=oh[:], in0=iota_f[:],
                                        scalar1=psidx_f[:],
                                        scalar2=None, op0=ALU.is_equal)

                # ---- counts / colsum -> [1, 20] ----
                ones = small.tile([128, 1], F32)
                nc.vector.memset(ones[:], 1.0)
                ps_cnt = ps_s.tile([1, 2 * NA], F32, tag="cnt")
                nc.tensor.matmul(ps_cnt[:, 0:NA], lhsT=ones[0:spc, :], rhs=oh[:],
                                 start=True, stop=True)
                nc.tensor.matmul(ps_cnt[:, NA:2 * NA], lhsT=ones[0:spc, :],
                                 rhs=sim[:], start=True, stop=True)
                ar_in = small.tile([1, 2 * NA], F32)
                nc.vector.tensor_copy(ar_in[:], ps_cnt[:])

            # ---- AllReduce of [1, 20] ----
            cc_in = dram.tile([1, 2 * NA], F32)
            cc_out = dram.tile([1, 2 * NA], F32, addr_space="Shared")
            nc.sync.dma_start(cc_in[:], ar_in[:])
            if no_ar:
                nc.sync.dma_start(cc_out[:], cc_in[:])
            else:
                nc.gpsimd.collective_compute(
                    "AllReduce", ALU.add,
                    replica_groups=[list(range(N_CORES))],
                    ins=[cc_in.opt()], outs=[cc_out.opt()],
                )
            ar = small.tile([1, 2 * NA], F32)
            nc.sync.dma_start(out=ar[:], in_=cc_out[:])

            # ---- majority vote (first max) + reduce_sim ----
            cmx = small.tile([1, 1], F32)
            nc.vector.tensor_reduce(out=cmx[:], in_=ar[:, 0:NA], axis=AX.X,
                                    op=ALU.max)
            cmask = small.tile([1, NA], I32)
            nc.vector.tensor_scalar(out=cmask[:], in0=ar[:, 0:NA], scalar1=cmx[:],
                                    scalar2=None, op0=ALU.is_ge)
            iota1 = small.tile([1, NA], I32)
            nc.gpsimd.iota(iota1[:], pattern=[[1, NA]], base=0,
                           channel_multiplier=0)
            iota1f = small.tile([1, NA], F32)
            nc.vector.tensor_copy(iota1f[:], iota1[:])
            big1 = small.tile([1, NA], I32)
            nc.vector.memset(big1[:], 127)
            csel = small.tile([1, NA], I32)
            nc.vector.select(csel[:], cmask[:], iota1[:], big1[:])
            major = small.tile([1, 1], I32)
            nc.vector.tensor_reduce(out=major[:], in_=csel[:], axis=AX.X,
                                    op=ALU.min)
            major_f = small.tile([1, 1], F32)
            nc.vector.tensor_copy(major_f[:], major[:])
            ohm = small.tile([1, NA], F32)
            nc.vector.tensor_scalar(out=ohm[:], in0=iota1f[:], scalar1=major_f[:],
                                    scalar2=None, op0=ALU.is_equal)
            rsp = small.tile([1, NA], F32)
            nc.vector.tensor_tensor(out=rsp[:], in0=ohm[:], in1=ar[:, NA:2 * NA],
                                    op=ALU.mult)
            rss = small.tile([1, 1], F32)
            nc.vector.tensor_reduce(out=rss[:], in_=rsp[:], axis=AX.X, op=ALU.add)
            scal = small.tile([1, 2], F32)
            nc.vector.tensor_copy(scal[:, 0:1], major_f[:])
            nc.vector.tensor_scalar_mul(scal[:, 1:2], rss[:],
                                        1.0 / (spc * N_CORES))
            nc.sync.dma_start(out=sc_ext[:], in_=scal[:])

            # ---- select W[major] arithmetically via one-hot ----
            with tc.tile_pool(name="ps_ob", bufs=1, space="PSUM") as ps_ob:
                ones_r = small.tile([1, 128], F32)
                nc.vector.memset(ones_r[:], 1.0)
                ohm_ps = ps_ob.tile([128, NA], F32)
                nc.tensor.matmul(ohm_ps[:], lhsT=ones_r[:], rhs=ohm[:],
                                 start=True, stop=True)
                ohm_b = small.tile([128, NA], F32)
                ohm_b_inst = nc.vector.tensor_copy(ohm_b[:], ohm_ps[:])
            if True:
                w1T = small.tile([128, NCH, H], BF16)
                nc.vector.tensor_scalar_mul(w1T[:], w1Ta[:, 0, :, :],
                                            ohm_b[:, 0:1])
                for a in range(1, NA):
                    nc.vector.scalar_tensor_tensor(
                        out=w1T[:], in0=w1Ta[:, a, :, :],
                        scalar=ohm_b[:, a:a + 1], in1=w1T[:],
                        op0=ALU.mult, op1=ALU.add)
                w2sel = small.tile([128, 6 * H], BF16)
                nc.vector.tensor_scalar_mul(w2sel[:], w2n[:, 0, :],
                                            ohm_b[:, 0:1])
                for a in range(1, NA):
                    nc.vector.scalar_tensor_tensor(
                        out=w2sel[:], in0=w2n[:, a, :],
                        scalar=ohm_b[:, a:a + 1], in1=w2sel[:],
                        op0=ALU.mult, op1=ALU.add)

                # ---- main pass, chunk-streamed:
                #      transpose x -> mm1 -> relu -> mm2 -> relu+residual ----
                with (
                    tc.tile_pool(name="xtc", bufs=6) as xtc,
                    tc.tile_pool(name="htc", bufs=2) as htc,
                    tc.tile_pool(name="ps_h", bufs=2, space="PSUM") as ps_h,
                    tc.tile_pool(name="ps_a", bufs=2, space="PSUM") as ps_a,
                    tc.tile_pool(name="ost", bufs=1) as ost,
                ):
                    v2 = small.tile([H, C], BF16)
                    vtp = ps_h.tile([H, C], BF16, tag="ph")
                    for q in range(6):
                        nc.tensor.transpose(vtp[:, ts(q, 128)],
                                            w2sel[:, ts(q, H)], ident[:])
                    # PSUM free order is (q, p); target c = 6p + q
                    nc.vector.tensor_copy(
                        v2[:].rearrange("h (p q) -> h q p", q=6),
                        vtp[:].rearrange("h (q p) -> h q p", p=128))
                    for k, (off, n) in enumerate(chunks):
                        nsub = n // 128
                        xTc = xtc.tile([128, NCH, 512], BF16, tag="xtc")
                        for ci in range(NCH):
                            xtp = ps_t.tile([128, 512], BF16, tag="tp", bufs=2)
                            for j in range(nsub):
                                t = off // 128 + j
                                nc.tensor.transpose(
                                    xtp[:, ts(j, 128)],
                                    x_nat[:, t, ts(ci, 128)], ident[:])
                            cpi = nc.vector.tensor_copy(
                                xTc[:, ci, 0:n], xtp[:, 0:n])
                            if k >= 4:
                                # let the vote/one-hot DVE ops preempt the
                                # transpose-copy backlog after the AllReduce
                                add_dep_helper(
                                    cpi.ins, ohm_b_inst.ins, sync=False,
                                    reason="xT copies yield to vote chain")
                        ph = ps_h.tile([H, 512], F32, tag="ph")
                        for ci in range(NCH):
                            nc.tensor.matmul(ph[:, 0:n], lhsT=w1T[:, ci, :],
                                             rhs=xTc[:, ci, 0:n],
                                             start=(ci == 0),
                                             stop=(ci == NCH - 1))
                        ht = htc.tile([H, 512], BF16, tag="ht")
                        nc.scalar.activation(out=ht[:, 0:n], in_=ph[:, 0:n],
                                             func=ACTF.Relu)
                        for j in range(nsub):
                            t = off // 128 + j
                            pa = ps_a.tile([128, C], F32, tag="pa")
                            nc.tensor.matmul(pa[:, 0:512],
                                             lhsT=ht[:, ts(j, 128)],
                                             rhs=v2[:, 0:512],
                                             start=True, stop=True)
                            nc.tensor.matmul(pa[:, 512:C],
                                             lhsT=ht[:, ts(j, 128)],
                                             rhs=v2[:, 512:C],
                                             start=True, stop=True)
                            ob = ost.tile([128, C], F32, tag="ob", bufs=6)
                            if t % 3 != 1:
                                nc.vector.scalar_tensor_tensor(
                                    out=ob[:], in0=pa[:], scalar=0.0,
                                    in1=x_nat[:, t, :],
                                    op0=ALU.max, op1=ALU.add)
                            else:
                                rel = ost.tile([128, C], F32, tag="rel", bufs=4)
                                nc.scalar.activation(out=rel[:], in_=pa[:],
                                                     func=ACTF.Relu)
                                nc.gpsimd.tensor_tensor(
                                    out=ob[:], in0=rel[:],
                                    in1=x_nat[:, t, :], op=ALU.add)
                            nr = 128 if t < nt - 1 else tail
                            eng = nc.sync if t % 2 == 0 else nc.scalar
                            eng.dma_start(
                                out=out_ext[t * 128:t * 128 + nr, :],
                                in_=ob[0:nr, :])


def build(spc=B // N_CORES, no_ar=False, reps=1):
    """Build the SPMD Bass program for `spc` samples per core.

    no_ar=True replaces the AllReduce with a local DMA (single-core
    timeline-sim debugging only — changes results across cores).
    reps>1 repeats the whole body (timing-slope measurement).
    """
    rows = spc * L
    nc = bacc.Bacc("TRN2", target_bir_lowering=False, debug=False,
                   num_devices=N_CORES)
    x_ext = nc.dram_tensor("x_embed", [rows, C], F32, kind="ExternalInput").ap()
    w1_ext = nc.dram_tensor("W1", [NA, H, C], F32, kind="ExternalInput").ap()
    w2_ext = nc.dram_tensor("W2", [NA, C, H], F32, kind="ExternalInput").ap()
    key_ext = nc.dram_tensor("adapter_key", [NA, C], F32,
                             kind="ExternalInput").ap()
    out_ext = nc.dram_tensor("out", [rows, C], F32, kind="ExternalOutput").ap()
    sc_ext = nc.dram_tensor("scalars", [1, 2], F32, kind="ExternalOutput").ap()

    with tile.TileContext(nc) as tc:
        for _ in range(reps):
            _emit_body(nc, tc, spc, no_ar, x_ext, w1_ext, w2_ext, key_ext,
                       out_ext, sc_ext)

    nc.compile()
    return nc


_built = {}


def _get_nc(spc):
    if spc not in _built:
        _built[spc] = build(spc)
    return _built[spc]


def kernel(x_embed, W1, W2, adapter_key):
    x_embed = np.ascontiguousarray(x_embed, dtype=np.float32)
    W1 = np.ascontiguousarray(W1, dtype=np.float32)
    W2 = np.ascontiguousarray(W2, dtype=np.float32)
    adapter_key = np.ascontiguousarray(adapter_key, dtype=np.float32)

    spc = B // N_CORES
    rows = spc * L
    nc = _get_nc(spc)
    shards = x_embed.reshape(N_CORES, rows, C)
    in_maps = [
        {"x_embed": shards[i], "W1": W1, "W2": W2, "adapter_key": adapter_key}
        for i in range(N_CORES)
    ]
    res = run_bass_kernel_spmd(nc, in_maps, list(range(N_CORES)))
    bias = np.concatenate(
        [res.results[i]["out"].reshape(1, spc, L, C) for i in range(N_CORES)],
        axis=0).reshape(B, L, C)
    scal = res.results[0]["scalars"]
    major = int(round(float(scal[0, 0])))
    reduce_sim = np.float32(scal[0, 1])
    idx = np.full((B, 1), major, dtype=np.int32)
    return bias, np.asarray(reduce_sim), idx


# revision 33
# speedup vs baseline: 21.9015x; 9.9018x over previous
"""Trainium2 Bass kernel for the moe_routing Adapter module.

Computes, SPMD over 8 NeuronCores (batch-sharded):
  x_mean  = mean_L(x); sim = cos(x_mean, adapter_key)  -> per-sample argmax
  majority vote over the 10-bin count vector (one small AllReduce)
  bias = x + relu(relu(x @ W1[major].T) @ W2[major].T);  reduce_sim; idx

kernel(**inputs) takes FULL inputs, returns (bias, reduce_sim, idx) like the
reference. Self-contained: only imports concourse (on PYTHONPATH) + numpy.

Implementation notes:
- x is cast-DMA'd f32->bf16 into SBUF once (natural layout) and kept resident;
  output writes stream back.  Total HBM traffic/core ~42MB (memory-bound).
- Per-sample means contract over rows, done as PE matmuls against per-tile
  0/1 indicator matrices (no transpose needed before the vote/AllReduce).
- x.T tiles (needed because mm1 contracts over the feature dim) are produced
  by PE transpose-mode chunk-by-chunk, overlapped with the AllReduce + mm1.
- No registers/dynamic DMA (value_load crashes this runtime): all 10 adapters
  are preloaded + pre-transposed; W[major] is selected arithmetically with a
  one-hot vector broadcast across partitions via a K=1 outer-product matmul.
"""

import numpy as np

import concourse.bacc as bacc
import concourse.tile as tile
import concourse.mybir as mybir
from concourse.bass import ts
from concourse.bass_utils import run_bass_kernel_spmd
from concourse.masks import make_identity
from concourse.tile_rust import add_dep_helper

N_CORES = 8
B, L, C = 256, 197, 768
H = 48
NA = 10
NCH = C // 128  # 6 column chunks

F32 = mybir.dt.float32
BF16 = mybir.dt.bfloat16
I32 = mybir.dt.int32
AX = mybir.AxisListType
ALU = mybir.AluOpType
ACTF = mybir.ActivationFunctionType


def _emit_body(nc, tc, spc, no_ar, x_ext, w1_ext, w2_ext, key_ext,
               out_ext, sc_ext):
    rows = spc * L
    nt = (rows + 127) // 128          # row tiles
    rpad = nt * 128
    tail = rows - (nt - 1) * 128      # valid rows in last tile
    chunks = [(o, min(512, rpad - o)) for o in range(0, rpad, 512)]

    if True:
        with (
            tc.tile_pool(name="big", bufs=1) as big,
            tc.tile_pool(name="small", bufs=1) as small,
            tc.tile_pool(name="ps_t", bufs=2, space="PSUM") as ps_t,
            tc.tile_pool(name="dram", bufs=1, space="DRAM") as dram,
        ):
            # ---- persistent SBUF tensors ----
            x_nat = big.tile([128, nt, C], BF16)        # natural layout, bf16
            w1Ta = big.tile([128, NA, NCH, H], BF16)    # W1[a].T chunks
            ident = small.tile([128, 128], BF16)
            ident10 = small.tile([NA, NA], F32)
            ident_s = small.tile([spc, spc], F32)
            make_identity(nc, ident)
            make_identity(nc, ident10)
            make_identity(nc, ident_s)

            # ---- all-10 adapter weights (issued before x: small, feeds
            #      PE transposes early; single contiguous cast DMAs) ----
            # W2[a] flattened: partition p holds rows 6p..6p+5 (c = 6p+q);
            # selected post-AR, then transposed (6 PE transposes).
            w2n = big.tile([128, NA, 6 * H], BF16)
            nc.gpsimd.dma_start(
                out=w2n[:],
                in_=w2_ext.rearrange("a (p q) h -> p a (q h)", p=128))
            with tc.tile_pool(name="wsta", bufs=1) as wsta:
                w1a = wsta.tile([H, NA, C], BF16)
                nc.gpsimd.dma_start(out=w1a[:],
                                    in_=w1_ext.rearrange("a h c -> h a c"))
                for a in range(NA):
                    wtp = ps_t.tile([128, NCH * H], BF16, tag="tp", bufs=2)
                    for ci in range(NCH):
                        nc.tensor.transpose(wtp[:, ts(ci, H)],
                                            w1a[:, a, ts(ci, 128)],
                                            ident[0:H, 0:H])
                    nc.vector.tensor_copy(
                        w1Ta[:, a, :, :].rearrange("p x h -> p (x h)"), wtp[:])

            # ---- load x (cast f32->bf16 via SWDGE), ~10 DMAs ----
            ntf = rows // 128  # full tiles
            group = max(1, (ntf + 9) // 10)
            for g0 in range(0, ntf, group):
                g1 = min(g0 + group, ntf)
                nc.gpsimd.dma_start(
                    out=x_nat[:, g0:g1, :],
                    in_=x_ext[g0 * 128:g1 * 128, :].rearrange(
                        "(t p) c -> p t c", p=128),
                )
            if tail < 128:
                nc.gpsimd.memset(x_nat[:, nt - 1, :], 0.0)
                nc.gpsimd.dma_start(
                    out=x_nat[0:tail, nt - 1, :],
                    in_=x_ext[(nt - 1) * 128:rows, :],
                )

            # ---- per-tile sample indicator matrices E[r, s] ----
            E = small.tile([128, nt, spc], BF16)
            nc.gpsimd.memset(E[:], 1.0)
            nc.gpsimd.affine_select(
                out=E[:], in_=E[:],
                pattern=[[128, nt], [-L, spc]], base=0, channel_multiplier=1,
                compare_op=ALU.is_ge, fill=0.0)
            nc.gpsimd.affine_select(
                out=E[:], in_=E[:],
                pattern=[[-128, nt], [L, spc]], base=L - 1,
                channel_multiplier=-1,
                compare_op=ALU.is_ge, fill=0.0)

            # ---- keys: load + l2-normalize + transpose ----
            keys = small.tile([NA, C], F32)
            nc.sync.dma_start(out=keys[:], in_=key_ext[:])
            ksq = small.tile([NA, C], F32)
            nc.vector.tensor_tensor(out=ksq[:], in0=keys[:], in1=keys[:],
                                    op=ALU.mult)
            kss = small.tile([NA, 1], F32)
            nc.vector.tensor_reduce(out=kss[:], in_=ksq[:], axis=AX.X, op=ALU.add)
            nc.vector.tensor_scalar_max(kss[:], kss[:], 1e-12)
            krt = small.tile([NA, 1], F32)
            nc.scalar.sqrt(krt[:], kss[:])
            kri = small.tile([NA, 1], F32)
            nc.vector.reciprocal(kri[:], krt[:])
            keyn = small.tile([NA, C], F32)
            nc.vector.tensor_scalar_mul(keyn[:], keys[:], kri[:])
            keynT = small.tile([128, NCH, NA], F32)
            for ci in range(NCH):
                ktp = ps_t.tile([128, NA], F32, tag="tp", bufs=2)
                nc.tensor.transpose(ktp[:], keyn[:, ts(ci, 128)], ident10[:])
                nc.vector.tensor_copy(keynT[:, ci, :], ktp[:])

            # ---- all-10 adapter weights: load (cast bf16) + transpose ----

            with (
                tc.tile_pool(name="ps_m", bufs=1, space="PSUM") as ps_m_pool,
                tc.tile_pool(name="ps_s", bufs=1, space="PSUM") as ps_s,
            ):
                # ---- per-sample sums: xsum[s, c] = sum_r E[r,s] x[r,c] ----
                ps_m = ps_m_pool.tile([spc, C], F32)
                for t in range(nt):
                    nc.tensor.matmul(ps_m[:, 0:512], lhsT=E[:, t, :],
                                     rhs=x_nat[:, t, 0:512],
                                     start=(t == 0), stop=(t == nt - 1))
                for t in range(nt):
                    nc.tensor.matmul(ps_m[:, 512:C], lhsT=E[:, t, :],
                                     rhs=x_nat[:, t, 512:C],
                                     start=(t == 0), stop=(t == nt - 1))
                xsum = small.tile([spc, C], F32)
                nc.vector.tensor_copy(xsum[:], ps_m[:])

                # ---- ||xsum||^2, 1/||xsum|| ----
                xsq = small.tile([spc, C], F32)
                nc.vector.tensor_tensor(out=xsq[:], in0=xsum[:], in1=xsum[:],
                                        op=ALU.mult)
                ssq = small.tile([spc, 1], F32)
                nc.vector.tensor_reduce(out=ssq[:], in_=xsq[:], axis=AX.X,
                                        op=ALU.add)
                nc.vector.tensor_scalar_max(ssq[:], ssq[:], 1e-9)
                srt = small.tile([spc, 1], F32)
                nc.scalar.sqrt(srt[:], ssq[:])
                sri = small.tile([spc, 1], F32)
                nc.vector.reciprocal(sri[:], srt[:])

                # ---- xsum.T chunks, then sim = (xsum @ keyn.T) / ||xsum|| ----
                xsumT = small.tile([128, NCH, spc], F32)
                for ci in range(NCH):
                    stp = ps_t.tile([128, spc], F32, tag="tp", bufs=2)
                    nc.tensor.transpose(stp[:], xsum[:, ts(ci, 128)], ident_s[:])
                    nc.vector.tensor_copy(xsumT[:, ci, :], stp[:])
                ps_sim = ps_s.tile([spc, NA], F32, tag="sim")
                for ci in range(NCH):
                    nc.tensor.matmul(ps_sim[:], lhsT=xsumT[:, ci, :],
                                     rhs=keynT[:, ci, :],
                                     start=(ci == 0), stop=(ci == NCH - 1))
                sim = small.tile([spc, NA], F32)
                nc.vector.tensor_scalar_mul(sim[:], ps_sim[:], sri[:])

                # ---- per-sample argmax (first max) -> one-hot ----
                smx = small.tile([spc, 1], F32)
                nc.vector.tensor_reduce(out=smx[:], in_=sim[:], axis=AX.X,
                                        op=ALU.max)
                mask = small.tile([spc, NA], I32)
                nc.vector.tensor_scalar(out=mask[:], in0=sim[:], scalar1=smx[:],
                                        scalar2=None, op0=ALU.is_ge)
                iota_s = small.tile([spc, NA], I32)
                nc.gpsimd.iota(iota_s[:], pattern=[[1, NA]], base=0,
                               channel_multiplier=0)
                iota_f = small.tile([spc, NA], F32)
                nc.vector.tensor_copy(iota_f[:], iota_s[:])
                bigc = small.tile([spc, NA], I32)
                nc.vector.memset(bigc[:], 127)
                msel = small.tile([spc, NA], I32)
                nc.vector.select(msel[:], mask[:], iota_s[:], bigc[:])
                psidx = small.tile([spc, 1], I32)
                nc.vector.tensor_reduce(out=psidx[:], in_=msel[:], axis=AX.X,
                                        op=ALU.min)
                psidx_f = small.tile([spc, 1], F32)
                nc.vector.tensor_copy(psidx_f[:], psidx[:])
                oh = small.tile([spc, NA], F32)
                nc.vector.tensor_scalar(out=oh[:], in0=iota_f[:],
                                        scalar1=psidx_f[:],
                                        scalar2=None, op0=ALU.is_equal)

                # ---- counts / colsum -> [1, 20] ----
                ones = small.tile([128, 1], F32)
                nc.vector.memset(ones[:], 1.0)
                ps_cnt = ps_s.tile([1, 2 * NA], F32, tag="cnt")
                nc.tensor.matmul(ps_cnt[:, 0:NA], lhsT=ones[0:spc, :], rhs=oh[:],
                                 start=True, stop=True)
                nc.tensor.matmul(ps_cnt[:, NA:2 * NA], lhsT=ones[0:spc, :],
                                 rhs=sim[:], start=True, stop=True)
                ar_in = small.tile([1, 2 * NA], F32)
                nc.vector.tensor_copy(ar_in[:], ps_cnt[:])

            # ---- AllReduce of [1, 20] ----
            cc_in = dram.tile([1, 2 * NA], F32)
            cc_out = dram.tile([1, 2 * NA], F32, addr_space="Shared")
            nc.sync.dma_start(cc_in[:], ar_in[:])
            if no_ar:
                nc.sync.dma_start(cc_out[:], cc_in[:])
            else:
                nc.gpsimd.collective_compute(
                    "AllReduce", ALU.add,
                    replica_groups=[list(range(N_CORES))],
                    ins=[cc_in.opt()], outs=[cc_out.opt()],
                )
            ar = small.tile([1, 2 * NA], F32)
            nc.sync.dma_start(out=ar[:], in_=cc_out[:])

            # ---- majority vote (first max) + reduce_sim ----
            cmx = small.tile([1, 1], F32)
            nc.vector.tensor_reduce(out=cmx[:], in_=ar[:, 0:NA], axis=AX.X,
                                    op=ALU.max)
            cmask = small.tile([1, NA], I32)
            nc.vector.tensor_scalar(out=cmask[:], in0=ar[:, 0:NA], scalar1=cmx[:],
                                    scalar2=None, op0=ALU.is_ge)
            iota1 = small.tile([1, NA], I32)
            nc.gpsimd.iota(iota1[:], pattern=[[1, NA]], base=0,
                           channel_multiplier=0)
            iota1f = small.tile([1, NA], F32)
            nc.vector.tensor_copy(iota1f[:], iota1[:])
            big1 = small.tile([1, NA], I32)
            nc.vector.memset(big1[:], 127)
            csel = small.tile([1, NA], I32)
            nc.vector.select(csel[:], cmask[:], iota1[:], big1[:])
            major = small.tile([1, 1], I32)
            nc.vector.tensor_reduce(out=major[:], in_=csel[:], axis=AX.X,
                                    op=ALU.min)
            major_f = small.tile([1, 1], F32)
            nc.vector.tensor_copy(major_f[:], major[:])
            ohm = small.tile([1, NA], F32)
            nc.vector.tensor_scalar(out=ohm[:], in0=iota1f[:], scalar1=major_f[:],
                                    scalar2=None, op0=ALU.is_equal)
            rsp = small.tile([1, NA], F32)
            nc.vector.tensor_tensor(out=rsp[:], in0=ohm[:], in1=ar[:, NA:2 * NA],
                                    op=ALU.mult)
            rss = small.tile([1, 1], F32)
            nc.vector.tensor_reduce(out=rss[:], in_=rsp[:], axis=AX.X, op=ALU.add)
            scal = small.tile([1, 2], F32)
            nc.vector.tensor_copy(scal[:, 0:1], major_f[:])
            nc.vector.tensor_scalar_mul(scal[:, 1:2], rss[:],
                                        1.0 / (spc * N_CORES))
            nc.sync.dma_start(out=sc_ext[:], in_=scal[:])

            # ---- select W[major] arithmetically via one-hot ----
            with tc.tile_pool(name="ps_ob", bufs=1, space="PSUM") as ps_ob:
                ones_r = small.tile([1, 128], F32)
                nc.vector.memset(ones_r[:], 1.0)
                ohm_ps = ps_ob.tile([128, NA], F32)
                nc.tensor.matmul(ohm_ps[:], lhsT=ones_r[:], rhs=ohm[:],
                                 start=True, stop=True)
                ohm_b = small.tile([128, NA], F32)
                ohm_b_inst = nc.vector.tensor_copy(ohm_b[:], ohm_ps[:])
            if True:
                w1T = small.tile([128, NCH, H], BF16)
                nc.vector.tensor_scalar_mul(w1T[:], w1Ta[:, 0, :, :],
                                            ohm_b[:, 0:1])
                for a in range(1, NA):
                    nc.vector.scalar_tensor_tensor(
                        out=w1T[:], in0=w1Ta[:, a, :, :],
                        scalar=ohm_b[:, a:a + 1], in1=w1T[:],
                        op0=ALU.mult, op1=ALU.add)
                w2sel = small.tile([128, 6 * H], BF16)
                nc.vector.tensor_scalar_mul(w2sel[:], w2n[:, 0, :],
                                            ohm_b[:, 0:1])
                for a in range(1, NA):
                    nc.vector.scalar_tensor_tensor(
                        out=w2sel[:], in0=w2n[:, a, :],
                        scalar=ohm_b[:, a:a + 1], in1=w2sel[:],
                        op0=ALU.mult, op1=ALU.add)

                # ---- main pass, chunk-streamed:
                #      transpose x -> mm1 -> relu -> mm2 -> relu+residual ----
                with (
                    tc.tile_pool(name="xtc", bufs=6) as xtc,
                    tc.tile_pool(name="htc", bufs=2) as htc,
                    tc.tile_pool(name="ps_h", bufs=2, space="PSUM") as ps_h,
                    tc.tile_pool(name="ps_a", bufs=2, space="PSUM") as ps_a,
                    tc.tile_pool(name="ost", bufs=1) as ost,
                ):
                    v2 = small.tile([H, C], BF16)
                    vtp = ps_h.tile([H, C], BF16, tag="ph")
                    for q in range(6):
                        nc.tensor.transpose(vtp[:, ts(q, 128)],
                                            w2sel[:, ts(q, H)], ident[:])
                    # PSUM free order is (q, p); target c = 6p + q
                    nc.vector.tensor_copy(
                        v2[:].rearrange("h (p q) -> h q p", q=6),
                        vtp[:].rearrange("h (q p) -> h q p", p=128))
                    for k, (off, n) in enumerate(chunks):
                        nsub = n // 128
                        xTc = xtc.tile([128, NCH, 512], BF16, tag="xtc")
                        for ci in range(NCH):
                            xtp = ps_t.tile([128, 512], BF16, tag="tp", bufs=2)
                            for j in range(nsub):
                                t = off // 128 + j
                                nc.tensor.transpose(
                                    xtp[:, ts(j, 128)],
                                    x_nat[:, t, ts(ci, 128)], ident[:])
                            cpi = nc.vector.tensor_copy(
                                xTc[:, ci, 0:n], xtp[:, 0:n])
                            if k >= 4:
                                # let the vote/one-hot DVE ops preempt the
                                # transpose-copy backlog after the AllReduce
                                add_dep_helper(
                                    cpi.ins, ohm_b_inst.ins, sync=False,
                                    reason="xT copies yield to vote chain")
                        ph = ps_h.tile([H, 512], F32, tag="ph")
                        for ci in range(NCH):
                            nc.tensor.matmul(ph[:, 0:n], lhsT=w1T[:, ci, :],
                                             rhs=xTc[:, ci, 0:n],
                                             start=(ci == 0),
                                             stop=(ci == NCH - 1))
                        ht = htc.tile([H, 512], BF16, tag="ht")
                        nc.scalar.activation(out=ht[:, 0:n], in_=ph[:, 0:n],
                                             func=ACTF.Relu)
                        for j in range(nsub):
                            t = off // 128 + j
                            pa = ps_a.tile([128, C], F32, tag="pa")
                            nc.tensor.matmul(pa[:, 0:512],
                                             lhsT=ht[:, ts(j, 128)],
                                             rhs=v2[:, 0:512],
                                             start=True, stop=True)
                            nc.tensor.matmul(pa[:, 512:C],
                                             lhsT=ht[:, ts(j, 128)],
                                             rhs=v2[:, 512:C],
                                             start=True, stop=True)
                            ob = ost.tile([128, C], F32, tag="ob", bufs=6)
                            if t % 3 != 1:
                                nc.vector.scalar_tensor_tensor(
                                    out=ob[:], in0=pa[:], scalar=0.0,
                                    in1=x_nat[:, t, :],
                                    op0=ALU.max, op1=ALU.add)
                            else:
                                rel = ost.tile([128, C], F32, tag="rel", bufs=4)
                                nc.scalar.activation(out=rel[:], in_=pa[:],
                                                     func=ACTF.Relu)
                                nc.gpsimd.tensor_tensor(
                                    out=ob[:], in0=rel[:],
                                    in1=x_nat[:, t, :], op=ALU.add)
                            nr = 128 if t < nt - 1 else tail
                            eng = nc.sync if t % 2 == 0 else nc.scalar
                            eng.dma_start(
                                out=out_ext[t * 128:t * 128 + nr, :],
                                in_=ob[0:nr, :])


def build(spc=B // N_CORES, no_ar=False, reps=1):
    """Build the SPMD Bass program for `spc` samples per core.

    no_ar=True replaces the AllReduce with a local DMA (single-core
    timeline-sim debugging only — changes results across cores).
    reps>1 repeats the whole body (timing-slope measurement).
    """
    rows = spc * L
    nc = bacc.Bacc("TRN2", target_bir_lowering=False, debug=False,
                   num_devices=N_CORES)
    x_ext = nc.dram_tensor("x_embed", [rows, C], F32, kind="ExternalInput").ap()
    w1_ext = nc.dram_tensor("W1", [NA, H, C], F32, kind="ExternalInput").ap()
    w2_ext = nc.dram_tensor("W2", [NA, C, H], F32, kind="ExternalInput").ap()
    key_ext = nc.dram_tensor("adapter_key", [NA, C], F32,
                             kind="ExternalInput").ap()
    out_ext = nc.dram_tensor("out", [rows, C], F32, kind="ExternalOutput").ap()
    sc_ext = nc.dram_tensor("scalars", [1, 2], F32, kind="ExternalOutput").ap()

    with tile.TileContext(nc) as tc:
        for _ in range(reps):
            _emit_body(nc, tc, spc, no_ar, x_ext, w1_ext, w2_ext, key_ext,
                       out_ext, sc_ext)

    nc.compile()
    return nc


_built = {}


def _get_nc(spc):
    if spc not in _built:
        _built[spc] = build(spc)
    return _built[spc]


def kernel(x_embed, W1, W2, adapter_key):
    x_embed = np.ascontiguousarray(x_embed, dtype=np.float32)
    W1 = np.ascontiguousarray(W1, dtype=np.float32)
    W2 = np.ascontiguousarray(W2, dtype=np.float32)
    adapter_key = np.ascontiguousarray(adapter_key, dtype=np.float32)

    spc = B // N_CORES
    rows = spc * L
    nc = _get_nc(spc)
    shards = x_embed.reshape(N_CORES, rows, C)
    in_maps = [
        {"x_embed": shards[i], "W1": W1, "W2": W2, "adapter_key": adapter_key}
        for i in range(N_CORES)
    ]
    res = run_bass_kernel_spmd(nc, in_maps, list(range(N_CORES)))
    bias = np.concatenate(
        [res.results[i]["out"].reshape(1, spc, L, C) for i in range(N_CORES)],
        axis=0).reshape(B, L, C)
    scal = res.results[0]["scalars"]
    major = int(round(float(scal[0, 0])))
    reduce_sim = np.float32(scal[0, 1])
    idx = np.full((B, 1), major, dtype=np.int32)
    return bias, np.asarray(reduce_sim), idx
